# revision 38
# baseline (speedup 1.0000x reference)
"""Trainium2 Bass kernel for nn_CrossCorrelation.

Reference (per batch b of 8, c=32 channels of 128x128):
  xs = standardize(x)  (zero mean, / (unbiased_std * sqrt(n)))
  Xf = fft2(xs); for ordered channel pairs (i, j>=i):
  cc = real(ifft2(Xf_i * conj(Xf_j))), rolled by (10,10), windowed 21x21.

Device algorithm (one batch per NeuronCore, 8 cores):
  - Input sent host-transposed [y, c, x] in bf16 (big DMA descriptors).
  - Standardization scale (1/(std*sqrt(n)) per channel) applied on HOST to
    the output (it is a per-pair constant factor); mean subtraction happens
    exactly on device via DC-bin zeroing.
  - y-FFT (stage A): one bf16 matmul per channel, moving [Fr|Fi] 256 cols.
  - x-FFT (stage B): bf16 matmuls per 7-channel group; DC bin zeroed.
  - Spectrum planes P1..P4 (bf16) feed 3-mult Gauss cross products.
  - Per-pair inverse: D^T[u,q] = sum_v m_t[v,u] * SM_t[v,q] with the
    PRODUCT PLANE as the matmul stationary (42-col moving constants);
    no PE transposes. Then out = gys^T @ D^T contracting u.
  - Output staged in SBUF, 4 chunked DMAs to HBM.
"""

import os
import numpy as np

H = W = 128
C = 32
B = 8
NPIX = H * W
MAX_S = 10
S = 2 * MAX_S + 1  # 21
NPAIR = C * (C + 1) // 2  # 528
NU = 65    # rfft bins along y
UPAD = 66  # u-stride in plane tensors (4B alignment for bf16)

GW = 24        # pairs per supergroup
NSG = NPAIR // GW  # 22

II, JJ = np.triu_indices(C)
BASE = np.zeros(C + 1, np.int64)
for _i in range(C):
    BASE[_i + 1] = BASE[_i] + (C - _i)


def _host_constants():
    import ml_dtypes

    k = np.arange(H)
    F = np.exp(-2j * np.pi * np.outer(k, k) / H)  # symmetric DFT matrix
    Fr = F.real
    Fi = F.imag
    # fc: [Fr | Fi | -Fi]; ffs = cols 0:256, FrS = 0:128, FiS = 128:256,
    # FinS = 256:384; ones col = col 0; ones row = row 0 of Fr.
    fc = np.concatenate([Fr, Fi, -Fi], axis=1).astype(np.float32)  # (128, 384)

    sy = (np.arange(S) - MAX_S) % H
    Gx = np.exp(2j * np.pi * np.outer(sy, np.arange(W)) / W) / NPIX  # (21,128)
    S1 = np.concatenate([Gx.real, Gx.imag], axis=0)   # (42, 128)
    S2 = np.concatenate([-Gx.imag, Gx.real], axis=0)
    u = np.arange(NU)
    Gy = np.exp(2j * np.pi * np.outer(sy, u) / H)  # (21, 65)
    w_u = np.ones(NU)
    w_u[1:64] = 2.0  # Hermitian fold weights for rfft-y
    Gyw = Gy * w_u

    sg = np.zeros((128, 560), np.float64)
    sg[:, 0:42] = S1.T
    sg[:, 42:84] = (S1 - S2).T
    sg[:, 84:126] = S2.T
    sg[0:NU, 126:147] = Gyw.real.T
    sg[0:NU, 147:168] = -Gyw.imag.T
    sg[:, 168] = 1.0  # bf16 ones column (spare)
    sg[:, 176:304] = Fr  # bf16 stage-A moving [Fr | Fi]; also stage-B stats
    sg[:, 304:432] = Fi
    sg[:, 432:560] = -Fi
    sg = sg.astype(ml_dtypes.bfloat16)

    return dict(sg=sg)


class _Balance:
    """Greedy engine load balancer (ns estimates per engine)."""

    def __init__(self):
        self.load = {"DVE": 0.0, "Act": 0.0, "Pool": 0.0}

    def pick(self, costs):
        e = min(costs, key=lambda k: self.load[k] + costs[k])
        self.load[e] += costs[e]
        return e


def build_nc():
    import concourse.bass as bass  # noqa: F401
    import concourse.mybir as mybir
    import concourse.tile as tile
    from concourse import bacc
    from contextlib import ExitStack

    f32 = mybir.dt.float32
    f32r = mybir.dt.float32r
    bf16 = mybir.dt.bfloat16
    AF = mybir.ActivationFunctionType
    ALU = mybir.AluOpType
    AX = mybir.AxisListType

    nc = bacc.Bacc("TRN2", target_bir_lowering=False, debug=False)

    x_d = nc.dram_tensor("x", [H, C, W], bf16, kind="ExternalInput").ap()
    sg_d = nc.dram_tensor("sg", [128, 560], bf16, kind="ExternalInput").ap()
    out_d = nc.dram_tensor("out", [NPAIR, S, S], f32, kind="ExternalOutput").ap()

    n = float(NPIX)
    bal = _Balance()

    with tile.TileContext(nc) as tc, ExitStack() as ctx:
        cpool = ctx.enter_context(tc.tile_pool(name="consts", bufs=1))
        spool = ctx.enter_context(tc.tile_pool(name="work", bufs=1))
        scrp = ctx.enter_context(tc.tile_pool(name="scr", bufs=2))

        sgt = cpool.tile([128, 560], bf16, tag="sg")

        gys = sgt[0:NU, 126:168]
        ffs = sgt[:, 176:432]
        FrS = sgt[:, 176:304]
        FiS = sgt[:, 304:432]
        FinS = sgt[:, 432:560]

        X = spool.tile([128, C, W], bf16, tag="X")
        T_s = spool.tile([128, 2, C, NU], bf16, tag="T")
        P1 = spool.tile([128, C, UPAD], bf16, tag="P1")
        P2 = spool.tile([128, C, UPAD], bf16, tag="P2")
        P3 = spool.tile([128, C, UPAD], bf16, tag="P3")
        P4 = spool.tile([128, C, UPAD], bf16, tag="P4")
        outbuf = spool.tile([S, NPAIR, S], f32, tag="outbuf")

        psB = tc.alloc_tile_pool(name="psB", bufs=1, space="PSUM")
        psA = tc.alloc_tile_pool(name="psA", bufs=int(os.environ.get("K_PSA", "6")),
                           space="PSUM")

        # ---------------- phase 1: input + FFTs ----------------
        # x in four chunks (high channels first); consts on the Act DGE queue
        nc.sync.dma_start(X[:, 24:32, :], x_d[:, 24:32, :])
        nc.scalar.dma_start(sgt[:, :], sg_d)
        nc.sync.dma_start(X[:, 16:24, :], x_d[:, 16:24, :])
        nc.sync.dma_start(X[:, 8:16, :], x_d[:, 8:16, :])
        nc.sync.dma_start(X[:, 0:8, :], x_d[:, 0:8, :])

        # stage A: 2 channels share one PSUM bank (tiles are bank-granular)
        NA0 = int(os.environ.get("K_NA0", "8"))
        pa_tiles = {}

        def emit_A(c_hi):
            # channels c_hi, c_hi-1 into one [128, 2, 2, 128] tile
            pa = psA.tile([128, 2, 2, 128], f32, tag="pa")
            for h, c in enumerate((c_hi, c_hi - 1)):
                nc.tensor.matmul(pa[:, h, :, :].rearrange("p a b -> p (a b)"),
                                 X[:, c, :], ffs, start=True, stop=True)
                pa_tiles[c] = (pa, h)

        def emit_T(c):
            pa, h = pa_tiles.pop(c)
            o = T_s[:, :, c, :]
            i = pa[:, h, :, 0:NU]
            e = bal.pick({"DVE": 260.0, "Act": 295.0})
            if e == "DVE":
                nc.vector.tensor_copy(o, i)
            else:
                nc.scalar.activation(o, i, AF.Copy)

        for c in range(C - 1, C - 1 - NA0, -2):
            emit_A(c)

        # stage B emitter
        BGROUPS = [(28, 4), (21, 7), (14, 7), (7, 7), (0, 7)]

        def emit_B(g0, w):
            br = psB.tile([128, 7, NU], f32, tag="br")
            bi = psB.tile([128, 7, NU], f32, tag="bi")
            TrT = T_s[:, 0, g0:g0 + w, :]
            TiT = T_s[:, 1, g0:g0 + w, :]
            nc.tensor.matmul(br[:, 0:w, :], FrS, TrT, start=True, stop=False)
            nc.tensor.matmul(br[:, 0:w, :], FinS, TiT, start=False, stop=True)
            nc.tensor.matmul(bi[:, 0:w, :], FiS, TrT, start=True, stop=False)
            nc.tensor.matmul(bi[:, 0:w, :], FrS, TiT, start=False, stop=True)
            # zero DC bin (u=0, v=0) == mean subtraction
            nc.vector.memset(br[0:1, 0:w, 0:1], 0.0)
            nc.vector.memset(bi[0:1, 0:w, 0:1], 0.0)
            gs = slice(g0, g0 + w)
            nc.scalar.activation(P4[:, gs, 0:NU], br[:, 0:w, :], AF.Copy)
            nc.scalar.activation(P2[:, gs, 0:NU], bi[:, 0:w, :], AF.Copy)
            bal.load["Act"] += 2 * 564.0
            nc.vector.tensor_tensor(P1[:, gs, 0:NU], P4[:, gs, 0:NU],
                                    P2[:, gs, 0:NU], op=ALU.add)
            nc.vector.tensor_tensor(P3[:, gs, 0:NU], P2[:, gs, 0:NU],
                                    P4[:, gs, 0:NU], op=ALU.subtract)
            bal.load["DVE"] += 2 * 297.0

        # ---------------- phase 2 pools (allocated later, after psA/psS) ----
        MB = int(os.environ.get("K_MB", "6"))
        mpool = dtpool = psDT = psO = None

        smv = [sgt[:, 0:42], sgt[:, 42:84], sgt[:, 84:126]]
        DMA_AFTER = {k: (NPAIR - GW * (k + 1), NPAIR - GW * (k - 1))
                     for k in range(1, NSG, 2)}

        def emit_sg(kk):
            plo = NPAIR - GW * (kk + 1)
            m1 = mpool.tile([128, GW, UPAD], bf16, tag="m1")
            m2 = mpool.tile([128, GW, UPAD], bf16, tag="m2")
            m3 = mpool.tile([128, GW, UPAD], bf16, tag="m3")
            # products, segmented by i-block (capped width for balance)
            s = 0
            while s < GW:
                p = plo + s
                i, j = int(II[p]), int(JJ[p])
                w = min(GW - s, int(BASE[i + 1]) - p, 12)
                bsh = [128, w, NU]
                dve_c = w * NU * 0.521 + 60.0
                pool_c = (w * NU * 1.984 + 95.0) * 1.12
                for m, Pa, Pb in ((m1, P1, P4), (m2, P2, P3), (m3, P3, P2)):
                    e = bal.pick({"DVE": dve_c, "Pool": pool_c})
                    eng = nc.vector if e == "DVE" else nc.gpsimd
                    eng.tensor_tensor(m[:, s:s + w, 0:NU],
                                      Pa[:, i:i + 1, 0:NU].broadcast_to(bsh),
                                      Pb[:, j:j + w, 0:NU], op=ALU.mult)
                s += w
            # D^T per pair: product plane as stationary, 42-col const moving
            dts = dtpool.tile([NU, GW, 42], bf16, tag="dts")
            for h in range(2):
                psdt = psDT.tile([NU, 12, 42], f32, tag="psdt")
                for t12 in range(12):
                    sl = 12 * h + t12
                    for t, m in enumerate((m1, m2, m3)):
                        nc.tensor.matmul(psdt[:, t12, :], m[:, sl, 0:NU],
                                         smv[t], start=(t == 0), stop=(t == 2))
                e = bal.pick({"DVE": 650.0, "Act": 605.0})
                if e == "DVE":
                    nc.vector.tensor_copy(dts[:, 12 * h:12 * h + 12, :],
                                          psdt[:, :, :])
                else:
                    nc.scalar.activation(dts[:, 12 * h:12 * h + 12, :],
                                         psdt[:, :, :], AF.Copy)
            pso = psO.tile([S, GW, S], f32, tag="pso")
            nc.tensor.matmul(pso[:, :, :], gys[:, 0:21], dts[:, :, 0:21],
                             start=True, stop=False)
            nc.tensor.matmul(pso[:, :, :], gys[:, 21:42], dts[:, :, 21:42],
                             start=False, stop=True)
            e = bal.pick({"DVE": 650.0, "Act": 605.0})
            if e == "DVE":
                nc.vector.tensor_copy(outbuf[:, plo:plo + GW, :], pso[:, :, :])
            else:
                nc.scalar.activation(outbuf[:, plo:plo + GW, :], pso[:, :, :],
                                     AF.Copy)
            if kk in DMA_AFTER:
                a, b = DMA_AFTER[kk]
                nc.sync.dma_start(out_d[a:b, :, :].transpose([1, 0, 2]),
                                  outbuf[:, a:b, :])

        # ------------- master emission sequence -------------
        for c in range(C - 1 - NA0, -1, -2):
            emit_A(c)
        for c in range(C - 1, -1, -1):
            emit_T(c)
        psA.release()
        bal.load = {k: 0.0 for k in bal.load}
        mpool = tc.alloc_tile_pool(name="mpool", bufs=MB)
        dtpool = tc.alloc_tile_pool(name="dtpool", bufs=4)
        psDT = tc.alloc_tile_pool(name="psDT", bufs=3, space="PSUM")
        psO = tc.alloc_tile_pool(name="psO", bufs=3, space="PSUM")
        emit_B(*BGROUPS[0])
        emit_B(*BGROUPS[1])
        emit_sg(0)
        emit_B(*BGROUPS[2])
        emit_sg(1)
        emit_sg(2)
        emit_B(*BGROUPS[3])
        emit_sg(3)
        emit_sg(4)
        emit_sg(5)
        emit_B(*BGROUPS[4])
        for kk in range(6, NSG):
            emit_sg(kk)
        psO.release()
        psDT.release()
        dtpool.release()
        mpool.release()
        psB.release()

    nc.compile()
    return nc


_CACHE = {}


def _get_nc():
    if "nc" not in _CACHE:
        _CACHE["nc"] = build_nc()
    return _CACHE["nc"]


TRACE = False  # test harness can flip this to capture a profile


def kernel(x: np.ndarray) -> np.ndarray:
    import ml_dtypes
    from concourse.bass_utils import run_bass_kernel_spmd

    assert x.shape == (B, C, H, W) and x.dtype == np.float32
    nc = _get_nc()
    consts = _host_constants()
    in_maps = []
    for b in range(B):
        m = {"x": np.ascontiguousarray(
            x[b].transpose(1, 0, 2)).astype(ml_dtypes.bfloat16)}
        m.update(consts)
        in_maps.append(m)
    res = run_bass_kernel_spmd(nc, in_maps, core_ids=list(range(B)), trace=TRACE)
    _CACHE["last_results"] = res
    out = np.stack([r["out"] for r in res.results]).astype(np.float32)
    # standardization scale: out_ij *= s_i * s_j with s = 1/(std*sqrt(n)),
    # matching the reference exactly (ddof=1, std<eps -> scale 0)
    xb = x.astype(ml_dtypes.bfloat16).astype(np.float32)  # device saw bf16
    std = xb.reshape(B, C, -1).std(axis=2, ddof=1)
    std = np.where(std < 1e-9, np.inf, std)
    sc = 1.0 / (std * np.sqrt(np.float32(NPIX)))  # [B, C]
    out *= (sc[:, II] * sc[:, JJ])[:, :, None, None]
    return out
